# revision 16
# baseline (speedup 1.0000x reference)
"""Causal self-attention kernel for 8 Trainium2 NeuronCores.

Problem: B=4, T=2048, C=1024, H=16 heads (D=64).
Sharding: data-parallel over batch (4) x tensor-parallel over heads (2 groups
of 8 heads). Core c handles batch c//2, head-group c%2. Each core computes
qkv for its 8 heads, full causal attention on TxT scores, and its partial
projection output; the host sums the two head-group partials per batch.

v2 layout/schedule notes (per core):
  - i-chunk-outer pipeline: for each 512-token chunk ic: load x chunk,
    compute v tiles + q/k for ALL head pairs, then attention for all pairs
    at i-chunk ic (causal j <= i), then deferred normalization for ic and
    the projection for chunk ic-1 (one chunk behind, so the PE never waits
    on the normalization chain).
  - causal mask applied as a 0/1 bf16 multiply on the exp() output of the
    diagonal j-tile (cheap DVE op in 2x mode) instead of a -1e30 PSUM add.
  - softmax denominator l comes from a ones-column in v (PV matmul row 64);
    l rows are staged to SBUF, lane-spread with one DMA per chunk,
    inverted with reciprocal_approx_fast, cast to bf16, partition-broadcast
    with stride-0-source DMAs, and multiplied into yU in bf16.
  - projection bias+copy runs on the Scalar engine (per-partition bias),
    keeping DVE free; all steady-state DMAs are issued on the sync queue so
    the Scalar sequencer never stalls the exp stream.
  - all matmuls run in bf16 with fp32 PSUM accumulation; score pairs use
    tile_position row-packing (concurrent K=64 matmuls).
"""

import sys

if "/opt/trn_rl_repo" not in sys.path:
    sys.path.insert(0, "/opt/trn_rl_repo")

from contextlib import ExitStack

import ml_dtypes
import numpy as np

import concourse.bass as bass
import concourse.mybir as mybir
import concourse.tile as tile
from concourse.bass_utils import run_bass_kernel_spmd

BF16 = mybir.dt.bfloat16
F32 = mybir.dt.float32
NP_BF16 = ml_dtypes.bfloat16

P = 128
B, T, C = 4, 2048, 1024
H = 16
D = 64
HL = 8            # heads per core
NPAIR = HL // 2   # head pairs per core
NL = HL * D       # 512: local qkv width
CT = C // P       # 8 contraction tiles over C
DT = NL // P      # 4 contraction tiles over local head dims
NTO = C // P      # 8 output tiles for proj
TCH = T // 512    # 4 t-chunks
NJT = T // P      # 16 j tiles


def _split_excess_waits(nc, limit=1):
    """This walrus build supports a single sem-wait per instruction; move
    excess waits emitted by Tile onto preceding same-engine NoOps."""
    n = 0
    for bb in nc.main_func.blocks:
        out = []
        changed = False
        for inst in bb.instructions:
            si = inst.sync_info
            if si is not None and len(si.on_wait) > limit:
                waits = list(si.on_wait)
                excess, keep = waits[:-limit], waits[-limit:]
                for i in range(0, len(excess), limit):
                    out.append(
                        mybir.InstNoOp(
                            name=f"waitsplit_{n}",
                            ins=[],
                            outs=[],
                            engine=inst.engine,
                            sync_info=mybir.SyncInfo(
                                on_wait=excess[i : i + limit], on_update=[]
                            ),
                        )
                    )
                    n += 1
                si.on_wait = keep
                changed = True
            out.append(inst)
        if changed:
            bb.instructions = out
    return n


def build_nc(split_waits=True):
    nc = bass.Bass()
    AF = mybir.ActivationFunctionType

    xT = nc.dram_tensor("xT", [P, TCH, CT, 512], BF16, kind="ExternalInput")
    wq = nc.dram_tensor("wq", [P, CT, NL], BF16, kind="ExternalInput")
    wk = nc.dram_tensor("wk", [P, CT, NL], BF16, kind="ExternalInput")
    wv = nc.dram_tensor("wv", [P, CT, NL], BF16, kind="ExternalInput")
    wp = nc.dram_tensor("wp", [P, DT, C], BF16, kind="ExternalInput")
    bq = nc.dram_tensor("bq", [P, NPAIR], F32, kind="ExternalInput")
    bk = nc.dram_tensor("bk", [P, NPAIR], F32, kind="ExternalInput")
    bv = nc.dram_tensor("bv", [P, NL], F32, kind="ExternalInput")
    bp = nc.dram_tensor("bp", [P, NTO], F32, kind="ExternalInput")
    msk = nc.dram_tensor("msk", [P, P], BF16, kind="ExternalInput")
    outT = nc.dram_tensor("outT", [P, NTO, T], F32, kind="ExternalOutput")

    with tile.TileContext(nc) as tc, ExitStack() as ctx:
        persist = ctx.enter_context(tc.tile_pool(name="persist", bufs=1))
        spsum = ctx.enter_context(tc.tile_pool(name="spsum", bufs=3, space="PSUM"))
        ypsum = ctx.enter_context(tc.tile_pool(name="ypsum", bufs=1, space="PSUM"))
        work = ctx.enter_context(tc.tile_pool(name="work", bufs=3))
        nwork = ctx.enter_context(tc.tile_pool(name="nwork", bufs=2))

        # ---- persistent SBUF tensors ----
        qT = persist.tile([P, NPAIR, T], BF16)   # [2x64d, pair, t]
        kT = persist.tile([P, NPAIR, T], BF16)
        vA = persist.tile([P, NJT, HL, D + 1], BF16)  # [j, jt, head, d|ones]
        yU = persist.tile([P, DT, T], BF16)  # y.T pair-packed; normalized in place
        lst = persist.tile([P, 2 * NPAIR, 512], F32)  # l staging (row 64 only)
        lrow = persist.tile([P, 512], F32)   # lane-spread l: row 8*ic+2*pr+h
        linv = persist.tile([P, 512], F32)
        linvb = persist.tile([P, 512], BF16)
        linv0 = persist.tile([1, 4 * 8, 512], BF16)  # 1/l rows gathered at p0
        onesb = persist.tile([1, D], BF16)   # lhsT for PE partition-broadcast
        xs = persist.tile([P, TCH, CT, 512], BF16)
        wqs = persist.tile([P, CT, NL], BF16)
        wks = persist.tile([P, CT, NL], BF16)
        wvs = persist.tile([P, CT, NL], BF16)
        wps = persist.tile([P, DT, C], BF16)
        bqs = persist.tile([P, NPAIR], F32)
        bks = persist.tile([P, NPAIR], F32)
        bvs = persist.tile([P, NL], F32)
        bps = persist.tile([P, NTO], F32)
        msks = persist.tile([P, 1, P], BF16)

        # ---- input DMAs: only what block 0 needs up front (x chunk 0 +
        # wv/wq/wk + biases); later chunks and wp are issued inside the
        # pipeline so they don't steal HBM bandwidth from the startup ----
        nc.scalar.dma_start(wvs[:], wv[:])
        nc.scalar.dma_start(bvs[:], bv[:])
        nc.scalar.dma_start(bqs[:], bq[:])
        nc.scalar.dma_start(bks[:], bk[:])
        nc.scalar.dma_start(msks[:, 0, :], msk[:])
        nc.scalar.dma_start(bps[:], bp[:])

        nc.sync.dma_start(xs[:, 0, 0:4, :], xT[:, 0, 0:4, :])
        nc.sync.dma_start(xs[:, 0, 4:8, :], xT[:, 0, 4:8, :])
        nc.sync.dma_start(wqs[:], wq[:])

        nc.gpsimd.dma_start(wks[:], wk[:])

        nc.vector.memset(vA[:, :, :, D : D + 1], 1.0)
        nc.vector.memset(onesb[:], 1.0)

        # warm the PE clock gate + preload the Exp table while DMAs run;
        # the warmup reads msks so it only waits on that tiny transfer
        wps_ = spsum.tile([P, 2, 512], F32, tag="s")
        nc.scalar.activation(lst[0:1, 0, 0:2], msks[0:1, 0, 0:2], AF.Exp,
                             scale=1.0)
        for _ in range(40):
            nc.tensor.matmul(wps_[:, 0, 0:P], lhsT=msks[:, 0, :],
                             rhs=msks[:, 0, :], start=True, stop=True)

        def xsl(ct, t0, n):  # slice of xs covering [t0, t0+n) at c-tile ct
            tc_i, o = divmod(t0, 512)
            return xs[:, tc_i, ct, o : o + n]

        def emit_v(tt):
            ps = spsum.tile([P, 2, 512], F32, tag="s")
            for ct in range(CT):
                nc.tensor.matmul(
                    ps[:, 0, :],
                    lhsT=xsl(ct, tt * P, P),
                    rhs=wvs[:, ct, :],
                    start=(ct == 0),
                    stop=(ct == CT - 1),
                )
            nc.vector.tensor_tensor(
                out=vA[:, tt, :, 0:D],
                in0=ps[:, 0, :].rearrange("p (h d) -> p h d", h=HL),
                in1=bvs.rearrange("p (h d) -> p h d", h=HL),
                op=mybir.AluOpType.add,
            )

        def emit_qk(nt, tc_i):
            ps = spsum.tile([P, 2, 512], F32, tag="s")
            t_sl = slice(tc_i * 512, (tc_i + 1) * 512)
            for ct in range(CT):
                nc.tensor.matmul(
                    ps[:, 0, :],
                    lhsT=wqs[:, ct, nt * P : (nt + 1) * P],
                    rhs=xs[:, tc_i, ct, :],
                    start=(ct == 0),
                    stop=(ct == CT - 1),
                )
            for ct in range(CT):
                nc.tensor.matmul(
                    ps[:, 1, :],
                    lhsT=wks[:, ct, nt * P : (nt + 1) * P],
                    rhs=xs[:, tc_i, ct, :],
                    start=(ct == 0),
                    stop=(ct == CT - 1),
                )
            nc.scalar.activation(
                qT[:, nt, t_sl], ps[:, 0, :], AF.Identity,
                bias=bqs[:, nt : nt + 1], scale=1.0,
            )
            nc.scalar.activation(
                kT[:, nt, t_sl], ps[:, 1, :], AF.Identity,
                bias=bks[:, nt : nt + 1], scale=1.0,
            )

        def emit_attention(pr, ic):
            hA, hB = 2 * pr, 2 * pr + 1
            njt = 4 * ic + 4  # causal: j tiles 0 .. 4*ic+3
            i0 = ic * 512
            yA = ypsum.tile([D + 1, 512], F32, tag="yA")
            yB = ypsum.tile([D + 1, 512], F32, tag="yB")
            sts = {}

            def emit_scores(jt):
                st = spsum.tile([P, 2, 512], F32, tag="s")
                sts[jt] = st
                ow = max(0, jt * P - i0)
                j_sl = slice(jt * P, (jt + 1) * P)
                i_sl = slice(i0 + ow, i0 + 512)
                nc.tensor.matmul(
                    st[:, 0, ow:512],
                    lhsT=kT[0:D, pr, j_sl],
                    rhs=qT[0:D, pr, i_sl],
                    start=True, stop=True,
                    tile_position=(0, 0),
                )
                nc.tensor.matmul(
                    st[:, 1, ow:512],
                    lhsT=kT[D:P, pr, j_sl],
                    rhs=qT[D:P, pr, i_sl],
                    start=True, stop=True,
                    tile_position=(64, 0),
                )

            emit_scores(0)
            if njt > 1:
                emit_scores(1)
            for jt in range(njt):
                st = sts.pop(jt)
                ow = max(0, jt * P - i0)
                pt = work.tile([P, 2, 512], BF16, tag="p")
                nc.scalar.activation(
                    pt[:, :, ow:512], st[:, :, ow:512], AF.Exp, scale=0.125
                )
                if jt >= 4 * ic:  # diagonal tile: zero above-diag via 0/1 mask
                    nc.vector.tensor_tensor(
                        out=pt[:, :, ow : ow + P],
                        in0=pt[:, :, ow : ow + P],
                        in1=msks[:].to_broadcast([P, 2, P]),
                        op=mybir.AluOpType.mult,
                    )
                if jt + 2 < njt:
                    emit_scores(jt + 2)
                nc.tensor.matmul(
                    yA[:, ow:512],
                    lhsT=vA[:, jt, hA, :],
                    rhs=pt[:, 0, ow:512],
                    start=(jt == 0),
                    stop=(jt == njt - 1),
                )
                nc.tensor.matmul(
                    yB[:, ow:512],
                    lhsT=vA[:, jt, hB, :],
                    rhs=pt[:, 1, ow:512],
                    start=(jt == 0),
                    stop=(jt == njt - 1),
                )
            # stage l rows (fp32, partition 64) and the unnormalized y (bf16)
            i_sl = slice(i0, i0 + 512)
            nc.vector.tensor_copy(lst[D : D + 1, 2 * pr, :], yA[D : D + 1, :])
            nc.vector.tensor_copy(lst[D : D + 1, 2 * pr + 1, :], yB[D : D + 1, :])
            nc.vector.tensor_copy(yU[0:D, pr, i_sl], yA[0:D, :])
            nc.vector.tensor_copy(yU[D:P, pr, i_sl], yB[0:D, :])

        def emit_spread(ic):
            # lane-spread the 8 staged l rows, invert, cast to bf16
            # (rows 32*ic.. to keep engine base partitions 32-aligned)
            r0 = 32 * ic
            nc.sync.dma_start(lrow[r0 : r0 + 8, :], lst[D : D + 1, :, :])
            nc.vector.reciprocal(linv[r0 : r0 + 8, :], lrow[r0 : r0 + 8, :])
            nc.vector.tensor_copy(linvb[r0 : r0 + 8, :], linv[r0 : r0 + 8, :])
            nc.sync.dma_start(
                linv0[0:1, 8 * ic : 8 * ic + 8, :], linvb[r0 : r0 + 8, :]
            )

        def emit_apply(ic):
            # PE (ones-column x row) partition-broadcasts 1/l into PSUM and
            # DVE multiplies y_u in place.
            i_sl = slice(ic * 512, (ic + 1) * 512)
            for pr in range(NPAIR):
                rA = 8 * ic + 2 * pr
                lbA = ypsum.tile([D, 512], F32, tag="yA")
                lbB = ypsum.tile([D, 512], F32, tag="yB")
                nc.tensor.matmul(
                    lbA[:], lhsT=onesb[:], rhs=linv0[0:1, rA, :],
                    start=True, stop=True,
                )
                nc.tensor.matmul(
                    lbB[:], lhsT=onesb[:], rhs=linv0[0:1, rA + 1, :],
                    start=True, stop=True,
                )
                nc.vector.tensor_tensor(
                    out=yU[0:D, pr, i_sl], in0=yU[0:D, pr, i_sl], in1=lbA[:],
                    op=mybir.AluOpType.mult,
                )
                nc.vector.tensor_tensor(
                    out=yU[D:P, pr, i_sl], in0=yU[D:P, pr, i_sl], in1=lbB[:],
                    op=mybir.AluOpType.mult,
                )

        def emit_proj(tc_i, nts=range(NTO), on_act=True):
            t_sl = slice(tc_i * 512, (tc_i + 1) * 512)
            for nt in nts:
                ps = spsum.tile([P, 2, 512], F32, tag="s")
                for dt in range(DT):
                    nc.tensor.matmul(
                        ps[:, 0, :],
                        lhsT=wps[:, dt, nt * P : (nt + 1) * P],
                        rhs=yU[:, dt, t_sl],
                        start=(dt == 0),
                        stop=(dt == DT - 1),
                    )
                ot = work.tile([P, 512], F32, tag="out")
                if on_act:
                    nc.scalar.activation(
                        ot[:], ps[:, 0, :], AF.Identity,
                        bias=bps[:, nt : nt + 1], scale=1.0,
                    )
                else:
                    nc.vector.tensor_scalar(
                        out=ot[:], in0=ps[:, 0, :],
                        scalar1=bps[:, nt : nt + 1], scalar2=None,
                        op0=mybir.AluOpType.add,
                    )
                nc.sync.dma_start(outT[:, nt, t_sl], ot[:])

        # ---- main pipeline: chunk-outer with fine-grained interleave.
        # Block ic emits attention for chunk ic alternating with v/qk for
        # chunk ic+1 (fills PE during exp waits, keeps the clock gate warm);
        # normalization-apply and proj run one chunk behind. ----
        nc.sync.dma_start(xs[:, 1, 0:4, :], xT[:, 1, 0:4, :])
        nc.sync.dma_start(xs[:, 1, 4:8, :], xT[:, 1, 4:8, :])
        for tt in range(4):
            emit_v(tt)
        for pr in range(NPAIR):
            emit_qk(pr, 0)
        for ic in range(TCH):
            nxt = ic + 1
            if ic == 0:  # prefetch chunk 2 + proj weights during block 0
                nc.sync.dma_start(xs[:, 2, 0:4, :], xT[:, 2, 0:4, :])
                nc.sync.dma_start(xs[:, 2, 4:8, :], xT[:, 2, 4:8, :])
                nc.gpsimd.dma_start(wps[:], wp[:])
            elif ic == 1:  # prefetch chunk 3 during block 1
                nc.gpsimd.dma_start(xs[:, 3, 0:4, :], xT[:, 3, 0:4, :])
                nc.gpsimd.dma_start(xs[:, 3, 4:8, :], xT[:, 3, 4:8, :])
            for pr in range(NPAIR):
                emit_attention(pr, ic)
                if nxt < TCH:
                    emit_v(4 * nxt + pr)
                    emit_qk(pr, nxt)
            if ic > 0:
                emit_apply(ic - 1)
                emit_proj(ic - 1)
            emit_spread(ic)
        emit_apply(TCH - 1)
        emit_proj(TCH - 1)

    if split_waits:
        _split_excess_waits(nc, 1)
    return nc


def shard_inputs(x, w_attn, b_attn, w_proj, b_proj):
    """Build the 8 per-core input dicts (core = 2*batch + head_group)."""
    x = np.asarray(x, dtype=np.float32)
    w_attn = np.asarray(w_attn, dtype=np.float32)
    b_attn = np.asarray(b_attn, dtype=np.float32)
    w_proj = np.asarray(w_proj, dtype=np.float32)
    b_proj = np.asarray(b_proj, dtype=np.float32)

    # 0/1 multiplicative causal mask for a diagonal 128x128 block of
    # S.T ([j, i]): 1 where j <= i, 0 above the diagonal.
    pp = np.arange(P)
    msk = np.where(pp[:, None] <= pp[None, :], 1.0, 0.0).astype(NP_BF16)

    def wtile(w2d, ncols):  # [C_rows, ncols] -> [P, rows//P, ncols] bf16
        r = w2d.shape[0]
        return np.ascontiguousarray(
            w2d.reshape(r // P, P, ncols).transpose(1, 0, 2)
        ).astype(NP_BF16)

    in_maps = []
    for core in range(8):
        b, hg = divmod(core, 2)
        q0 = hg * NL
        xt = np.ascontiguousarray(x[b].T)  # [C, T]
        m = {
            "xT": np.ascontiguousarray(
                xt.reshape(CT, P, TCH, 512).transpose(1, 2, 0, 3)
            ).astype(NP_BF16),
            "wq": wtile(w_attn[:, q0 : q0 + NL], NL),
            "wk": wtile(w_attn[:, C + q0 : C + q0 + NL], NL),
            "wv": wtile(w_attn[:, 2 * C + q0 : 2 * C + q0 + NL], NL),
            "wp": wtile(w_proj[q0 : q0 + NL, :], C),
            "bq": np.ascontiguousarray(
                b_attn[q0 : q0 + NL].reshape(NPAIR, P).T
            ).astype(np.float32),
            "bk": np.ascontiguousarray(
                b_attn[C + q0 : C + q0 + NL].reshape(NPAIR, P).T
            ).astype(np.float32),
            "bv": np.broadcast_to(
                b_attn[2 * C + q0 : 2 * C + q0 + NL], (P, NL)
            ).astype(np.float32),
            "bp": (
                np.ascontiguousarray(b_proj.reshape(NTO, P).T).astype(np.float32)
                if hg == 0
                else np.zeros((P, NTO), np.float32)
            ),
            "msk": msk,
        }
        in_maps.append(m)
    return in_maps


def unshard_output(results):
    """Combine 8 per-core outT [P, NTO, T] partials into [B, T, C] fp32."""
    out = np.empty((B, T, C), dtype=np.float32)
    for b in range(B):
        acc = results[2 * b]["outT"] + results[2 * b + 1]["outT"]
        # [P, NTO, T] -> [C, T] -> [T, C]
        out[b] = acc.transpose(1, 0, 2).reshape(C, T).T
    return out


_NC_CACHE = {}


def kernel(x, w_attn, b_attn, w_proj, b_proj):
    if "nc" not in _NC_CACHE:
        _NC_CACHE["nc"] = build_nc()
    nc = _NC_CACHE["nc"]
    in_maps = shard_inputs(x, w_attn, b_attn, w_proj, b_proj)
    res = run_bass_kernel_spmd(nc, in_maps, core_ids=list(range(8)))
    return unshard_output(res.results)


# revision 17
# speedup vs baseline: 1.0669x; 1.0669x over previous
"""Causal self-attention kernel for 8 Trainium2 NeuronCores.

Problem: B=4, T=2048, C=1024, H=16 heads (D=64).
Sharding: data-parallel over batch (4) x tensor-parallel over heads (2 groups
of 8 heads). Core c handles batch c//2, head-group c%2. Each core computes
qkv for its 8 heads, full causal attention on TxT scores, and its partial
projection output; the host sums the two head-group partials per batch.

v2 layout/schedule notes (per core):
  - i-chunk-outer pipeline: for each 512-token chunk ic: load x chunk,
    compute v tiles + q/k for ALL head pairs, then attention for all pairs
    at i-chunk ic (causal j <= i), then deferred normalization for ic and
    the projection for chunk ic-1 (one chunk behind, so the PE never waits
    on the normalization chain).
  - causal mask applied as a 0/1 bf16 multiply on the exp() output of the
    diagonal j-tile (cheap DVE op in 2x mode) instead of a -1e30 PSUM add.
  - softmax denominator l comes from a ones-column in v (PV matmul row 64);
    l rows are staged to SBUF, lane-spread with one DMA per chunk,
    inverted with reciprocal_approx_fast, cast to bf16, partition-broadcast
    with stride-0-source DMAs, and multiplied into yU in bf16.
  - projection bias+copy runs on the Scalar engine (per-partition bias),
    keeping DVE free; all steady-state DMAs are issued on the sync queue so
    the Scalar sequencer never stalls the exp stream.
  - all matmuls run in bf16 with fp32 PSUM accumulation; score pairs use
    tile_position row-packing (concurrent K=64 matmuls).
"""

import sys

if "/opt/trn_rl_repo" not in sys.path:
    sys.path.insert(0, "/opt/trn_rl_repo")

from contextlib import ExitStack

import ml_dtypes
import numpy as np

import concourse.bass as bass
import concourse.mybir as mybir
import concourse.tile as tile
from concourse.bass_utils import run_bass_kernel_spmd

BF16 = mybir.dt.bfloat16
F32 = mybir.dt.float32
NP_BF16 = ml_dtypes.bfloat16

P = 128
B, T, C = 4, 2048, 1024
H = 16
D = 64
HL = 8            # heads per core
NPAIR = HL // 2   # head pairs per core
NL = HL * D       # 512: local qkv width
CT = C // P       # 8 contraction tiles over C
DT = NL // P      # 4 contraction tiles over local head dims
NTO = C // P      # 8 output tiles for proj
TCH = T // 512    # 4 t-chunks
NJT = T // P      # 16 j tiles


def _split_excess_waits(nc, limit=1):
    """This walrus build supports a single sem-wait per instruction; move
    excess waits emitted by Tile onto preceding same-engine NoOps."""
    n = 0
    for bb in nc.main_func.blocks:
        out = []
        changed = False
        for inst in bb.instructions:
            si = inst.sync_info
            if si is not None and len(si.on_wait) > limit:
                waits = list(si.on_wait)
                excess, keep = waits[:-limit], waits[-limit:]
                for i in range(0, len(excess), limit):
                    out.append(
                        mybir.InstNoOp(
                            name=f"waitsplit_{n}",
                            ins=[],
                            outs=[],
                            engine=inst.engine,
                            sync_info=mybir.SyncInfo(
                                on_wait=excess[i : i + limit], on_update=[]
                            ),
                        )
                    )
                    n += 1
                si.on_wait = keep
                changed = True
            out.append(inst)
        if changed:
            bb.instructions = out
    return n


def build_nc(split_waits=True):
    nc = bass.Bass()
    AF = mybir.ActivationFunctionType

    xT = nc.dram_tensor("xT", [P, TCH, CT, 512], BF16, kind="ExternalInput")
    wq = nc.dram_tensor("wq", [P, CT, NL], BF16, kind="ExternalInput")
    wk = nc.dram_tensor("wk", [P, CT, NL], BF16, kind="ExternalInput")
    wv = nc.dram_tensor("wv", [P, CT, NL], BF16, kind="ExternalInput")
    wp = nc.dram_tensor("wp", [P, DT, C], BF16, kind="ExternalInput")
    bq = nc.dram_tensor("bq", [P, NPAIR], F32, kind="ExternalInput")
    bk = nc.dram_tensor("bk", [P, NPAIR], F32, kind="ExternalInput")
    bv = nc.dram_tensor("bv", [P, NL], F32, kind="ExternalInput")
    bp = nc.dram_tensor("bp", [P, NTO], F32, kind="ExternalInput")
    msk = nc.dram_tensor("msk", [P, P], BF16, kind="ExternalInput")
    outT = nc.dram_tensor("outT", [P, NTO, T], BF16, kind="ExternalOutput")

    with tile.TileContext(nc) as tc, ExitStack() as ctx:
        persist = ctx.enter_context(tc.tile_pool(name="persist", bufs=1))
        spsum = ctx.enter_context(tc.tile_pool(name="spsum", bufs=2, space="PSUM"))
        ypsum = ctx.enter_context(tc.tile_pool(name="ypsum", bufs=2, space="PSUM"))
        work = ctx.enter_context(tc.tile_pool(name="work", bufs=3))
        nwork = ctx.enter_context(tc.tile_pool(name="nwork", bufs=2))

        # ---- persistent SBUF tensors ----
        qT = persist.tile([P, NPAIR, T], BF16)   # [2x64d, pair, t]
        kT = persist.tile([P, NPAIR, T], BF16)
        vA = persist.tile([P, NJT, HL, D + 1], BF16)  # [j, jt, head, d|ones]
        yU = persist.tile([P, DT, T], BF16)  # y.T pair-packed; normalized in place
        lst = persist.tile([P, 2 * NPAIR, 512], F32)  # l staging (row 64 only)
        lrow = persist.tile([P, 512], F32)   # lane-spread l: row 8*ic+2*pr+h
        linv = persist.tile([P, 512], F32)
        linvb = persist.tile([P, 512], BF16)
        linv0 = persist.tile([1, 4 * 8, 512], BF16)  # 1/l rows gathered at p0
        onesb = persist.tile([1, D], BF16)   # lhsT for PE partition-broadcast
        xs = persist.tile([P, TCH, CT, 512], BF16)
        wqs = persist.tile([P, CT, NL], BF16)
        wks = persist.tile([P, CT, NL], BF16)
        wvs = persist.tile([P, CT, NL], BF16)
        wps = persist.tile([P, DT, C], BF16)
        bqs = persist.tile([P, NPAIR], F32)
        bks = persist.tile([P, NPAIR], F32)
        bvs = persist.tile([P, NL], F32)
        bps = persist.tile([P, NTO], F32)
        msks = persist.tile([P, 1, P], BF16)

        # ---- input DMAs: only what block 0 needs up front (x chunk 0 +
        # wv/wq/wk + biases); later chunks and wp are issued inside the
        # pipeline so they don't steal HBM bandwidth from the startup ----
        nc.scalar.dma_start(wvs[:], wv[:])
        nc.scalar.dma_start(bvs[:], bv[:])
        nc.scalar.dma_start(bqs[:], bq[:])
        nc.scalar.dma_start(bks[:], bk[:])
        nc.scalar.dma_start(msks[:, 0, :], msk[:])
        nc.scalar.dma_start(bps[:], bp[:])

        nc.sync.dma_start(xs[:, 0, 0:4, :], xT[:, 0, 0:4, :])
        nc.sync.dma_start(xs[:, 0, 4:8, :], xT[:, 0, 4:8, :])
        nc.sync.dma_start(wqs[:], wq[:])

        nc.gpsimd.dma_start(wks[:], wk[:])

        nc.vector.memset(vA[:, :, :, D : D + 1], 1.0)
        nc.vector.memset(onesb[:], 1.0)

        # warm the PE clock gate + preload the Exp table while DMAs run;
        # the warmup reads msks so it only waits on that tiny transfer
        wps_ = spsum.tile([P, 2, 512], F32, tag="s")
        nc.scalar.activation(lst[0:1, 0, 0:2], msks[0:1, 0, 0:2], AF.Exp,
                             scale=1.0)
        for _ in range(40):
            nc.tensor.matmul(wps_[:, 0, 0:P], lhsT=msks[:, 0, :],
                             rhs=msks[:, 0, :], start=True, stop=True)

        def xsl(ct, t0, n):  # slice of xs covering [t0, t0+n) at c-tile ct
            tc_i, o = divmod(t0, 512)
            return xs[:, tc_i, ct, o : o + n]

        def emit_v(tt):
            ps = spsum.tile([P, 2, 512], F32, tag="s")
            for ct in range(CT):
                nc.tensor.matmul(
                    ps[:, 0, :],
                    lhsT=xsl(ct, tt * P, P),
                    rhs=wvs[:, ct, :],
                    start=(ct == 0),
                    stop=(ct == CT - 1),
                )
            nc.vector.tensor_tensor(
                out=vA[:, tt, :, 0:D],
                in0=ps[:, 0, :].rearrange("p (h d) -> p h d", h=HL),
                in1=bvs.rearrange("p (h d) -> p h d", h=HL),
                op=mybir.AluOpType.add,
            )

        def emit_qk(nt, tc_i):
            ps = spsum.tile([P, 2, 512], F32, tag="s")
            t_sl = slice(tc_i * 512, (tc_i + 1) * 512)
            for ct in range(CT):
                nc.tensor.matmul(
                    ps[:, 0, :],
                    lhsT=wqs[:, ct, nt * P : (nt + 1) * P],
                    rhs=xs[:, tc_i, ct, :],
                    start=(ct == 0),
                    stop=(ct == CT - 1),
                )
            for ct in range(CT):
                nc.tensor.matmul(
                    ps[:, 1, :],
                    lhsT=wks[:, ct, nt * P : (nt + 1) * P],
                    rhs=xs[:, tc_i, ct, :],
                    start=(ct == 0),
                    stop=(ct == CT - 1),
                )
            nc.scalar.activation(
                qT[:, nt, t_sl], ps[:, 0, :], AF.Identity,
                bias=bqs[:, nt : nt + 1], scale=1.0,
            )
            nc.scalar.activation(
                kT[:, nt, t_sl], ps[:, 1, :], AF.Identity,
                bias=bks[:, nt : nt + 1], scale=1.0,
            )

        def emit_attention(pr, ic):
            hA, hB = 2 * pr, 2 * pr + 1
            njt = 4 * ic + 4  # causal: j tiles 0 .. 4*ic+3
            i0 = ic * 512
            yA = ypsum.tile([D + 1, 512], F32, tag="yA")
            yB = ypsum.tile([D + 1, 512], F32, tag="yB")
            sts = {}

            def emit_scores(jt):
                st = spsum.tile([P, 2, 512], F32, tag="s")
                sts[jt] = st
                ow = max(0, jt * P - i0)
                j_sl = slice(jt * P, (jt + 1) * P)
                i_sl = slice(i0 + ow, i0 + 512)
                nc.tensor.matmul(
                    st[:, 0, ow:512],
                    lhsT=kT[0:D, pr, j_sl],
                    rhs=qT[0:D, pr, i_sl],
                    start=True, stop=True,
                    tile_position=(0, 0),
                )
                nc.tensor.matmul(
                    st[:, 1, ow:512],
                    lhsT=kT[D:P, pr, j_sl],
                    rhs=qT[D:P, pr, i_sl],
                    start=True, stop=True,
                    tile_position=(64, 0),
                )

            emit_scores(0)
            if njt > 1:
                emit_scores(1)
            for jt in range(njt):
                st = sts.pop(jt)
                ow = max(0, jt * P - i0)
                pt = work.tile([P, 2, 512], BF16, tag="p")
                nc.scalar.activation(
                    pt[:, :, ow:512], st[:, :, ow:512], AF.Exp, scale=0.125
                )
                if jt >= 4 * ic:  # diagonal tile: zero above-diag via 0/1 mask
                    nc.vector.tensor_tensor(
                        out=pt[:, :, ow : ow + P],
                        in0=pt[:, :, ow : ow + P],
                        in1=msks[:].to_broadcast([P, 2, P]),
                        op=mybir.AluOpType.mult,
                    )
                if jt + 2 < njt:
                    emit_scores(jt + 2)
                nc.tensor.matmul(
                    yA[:, ow:512],
                    lhsT=vA[:, jt, hA, :],
                    rhs=pt[:, 0, ow:512],
                    start=(jt == 0),
                    stop=(jt == njt - 1),
                )
                nc.tensor.matmul(
                    yB[:, ow:512],
                    lhsT=vA[:, jt, hB, :],
                    rhs=pt[:, 1, ow:512],
                    start=(jt == 0),
                    stop=(jt == njt - 1),
                )
            # stage l rows (fp32, partition 64) and the unnormalized y (bf16)
            i_sl = slice(i0, i0 + 512)
            nc.vector.tensor_copy(lst[D : D + 1, 2 * pr, :], yA[D : D + 1, :])
            nc.vector.tensor_copy(lst[D : D + 1, 2 * pr + 1, :], yB[D : D + 1, :])
            nc.vector.tensor_copy(yU[0:D, pr, i_sl], yA[0:D, :])
            nc.vector.tensor_copy(yU[D:P, pr, i_sl], yB[0:D, :])

        def emit_spread(ic):
            # lane-spread the 8 staged l rows, invert, cast to bf16
            # (rows 32*ic.. to keep engine base partitions 32-aligned)
            r0 = 32 * ic
            nc.sync.dma_start(lrow[r0 : r0 + 8, :], lst[D : D + 1, :, :])
            nc.vector.reciprocal(linv[r0 : r0 + 8, :], lrow[r0 : r0 + 8, :])
            nc.vector.tensor_copy(linvb[r0 : r0 + 8, :], linv[r0 : r0 + 8, :])
            nc.sync.dma_start(
                linv0[0:1, 8 * ic : 8 * ic + 8, :], linvb[r0 : r0 + 8, :]
            )

        def emit_apply(ic):
            # PE (ones-column x row) partition-broadcasts 1/l into PSUM and
            # DVE multiplies y_u in place.
            i_sl = slice(ic * 512, (ic + 1) * 512)
            for pr in range(NPAIR):
                rA = 8 * ic + 2 * pr
                lbA = ypsum.tile([D, 512], F32, tag="yA")
                lbB = ypsum.tile([D, 512], F32, tag="yB")
                nc.tensor.matmul(
                    lbA[:], lhsT=onesb[:], rhs=linv0[0:1, rA, :],
                    start=True, stop=True,
                )
                nc.tensor.matmul(
                    lbB[:], lhsT=onesb[:], rhs=linv0[0:1, rA + 1, :],
                    start=True, stop=True,
                )
                nc.vector.tensor_tensor(
                    out=yU[0:D, pr, i_sl], in0=yU[0:D, pr, i_sl], in1=lbA[:],
                    op=mybir.AluOpType.mult,
                )
                nc.vector.tensor_tensor(
                    out=yU[D:P, pr, i_sl], in0=yU[D:P, pr, i_sl], in1=lbB[:],
                    op=mybir.AluOpType.mult,
                )

        def emit_proj(tc_i, nts=range(NTO), on_act=True):
            t_sl = slice(tc_i * 512, (tc_i + 1) * 512)
            for nt in nts:
                ps = spsum.tile([P, 2, 512], F32, tag="s")
                for dt in range(DT):
                    nc.tensor.matmul(
                        ps[:, 0, :],
                        lhsT=wps[:, dt, nt * P : (nt + 1) * P],
                        rhs=yU[:, dt, t_sl],
                        start=(dt == 0),
                        stop=(dt == DT - 1),
                    )
                ot = work.tile([P, 512], BF16, tag="out")
                if on_act:
                    nc.scalar.activation(
                        ot[:], ps[:, 0, :], AF.Identity,
                        bias=bps[:, nt : nt + 1], scale=1.0,
                    )
                else:
                    nc.vector.tensor_scalar(
                        out=ot[:], in0=ps[:, 0, :],
                        scalar1=bps[:, nt : nt + 1], scalar2=None,
                        op0=mybir.AluOpType.add,
                    )
                nc.sync.dma_start(outT[:, nt, t_sl], ot[:])

        # ---- main pipeline: chunk-outer with fine-grained interleave.
        # Block ic emits attention for chunk ic alternating with v/qk for
        # chunk ic+1 (fills PE during exp waits, keeps the clock gate warm);
        # normalization-apply and proj run one chunk behind. ----
        nc.sync.dma_start(xs[:, 1, 0:4, :], xT[:, 1, 0:4, :])
        nc.sync.dma_start(xs[:, 1, 4:8, :], xT[:, 1, 4:8, :])
        for tt in range(4):
            emit_v(tt)
        for pr in range(NPAIR):
            emit_qk(pr, 0)
        for ic in range(TCH):
            nxt = ic + 1
            if ic == 0:  # prefetch chunk 2 + proj weights during block 0
                nc.sync.dma_start(xs[:, 2, 0:4, :], xT[:, 2, 0:4, :])
                nc.sync.dma_start(xs[:, 2, 4:8, :], xT[:, 2, 4:8, :])
                nc.gpsimd.dma_start(wps[:], wp[:])
            elif ic == 1:  # prefetch chunk 3 during block 1
                nc.gpsimd.dma_start(xs[:, 3, 0:4, :], xT[:, 3, 0:4, :])
                nc.gpsimd.dma_start(xs[:, 3, 4:8, :], xT[:, 3, 4:8, :])
            for pr in range(NPAIR):
                emit_attention(pr, ic)
                if nxt < TCH:
                    emit_v(4 * nxt + pr)
                    emit_qk(pr, nxt)
            if ic > 0:
                emit_apply(ic - 1)
                emit_proj(ic - 1)
            emit_spread(ic)
        emit_apply(TCH - 1)
        emit_proj(TCH - 1)

    if split_waits:
        _split_excess_waits(nc, 1)
    return nc


def shard_inputs(x, w_attn, b_attn, w_proj, b_proj):
    """Build the 8 per-core input dicts (core = 2*batch + head_group)."""
    x = np.asarray(x, dtype=np.float32)
    w_attn = np.asarray(w_attn, dtype=np.float32)
    b_attn = np.asarray(b_attn, dtype=np.float32)
    w_proj = np.asarray(w_proj, dtype=np.float32)
    b_proj = np.asarray(b_proj, dtype=np.float32)

    # 0/1 multiplicative causal mask for a diagonal 128x128 block of
    # S.T ([j, i]): 1 where j <= i, 0 above the diagonal.
    pp = np.arange(P)
    msk = np.where(pp[:, None] <= pp[None, :], 1.0, 0.0).astype(NP_BF16)

    def wtile(w2d, ncols):  # [C_rows, ncols] -> [P, rows//P, ncols] bf16
        r = w2d.shape[0]
        return np.ascontiguousarray(
            w2d.reshape(r // P, P, ncols).transpose(1, 0, 2)
        ).astype(NP_BF16)

    in_maps = []
    for core in range(8):
        b, hg = divmod(core, 2)
        q0 = hg * NL
        xt = np.ascontiguousarray(x[b].T)  # [C, T]
        m = {
            "xT": np.ascontiguousarray(
                xt.reshape(CT, P, TCH, 512).transpose(1, 2, 0, 3)
            ).astype(NP_BF16),
            "wq": wtile(w_attn[:, q0 : q0 + NL], NL),
            "wk": wtile(w_attn[:, C + q0 : C + q0 + NL], NL),
            "wv": wtile(w_attn[:, 2 * C + q0 : 2 * C + q0 + NL], NL),
            "wp": wtile(w_proj[q0 : q0 + NL, :], C),
            "bq": np.ascontiguousarray(
                b_attn[q0 : q0 + NL].reshape(NPAIR, P).T
            ).astype(np.float32),
            "bk": np.ascontiguousarray(
                b_attn[C + q0 : C + q0 + NL].reshape(NPAIR, P).T
            ).astype(np.float32),
            "bv": np.broadcast_to(
                b_attn[2 * C + q0 : 2 * C + q0 + NL], (P, NL)
            ).astype(np.float32),
            "bp": (
                np.ascontiguousarray(b_proj.reshape(NTO, P).T).astype(np.float32)
                if hg == 0
                else np.zeros((P, NTO), np.float32)
            ),
            "msk": msk,
        }
        in_maps.append(m)
    return in_maps


def unshard_output(results):
    """Combine 8 per-core outT [P, NTO, T] partials into [B, T, C] fp32."""
    out = np.empty((B, T, C), dtype=np.float32)
    for b in range(B):
        acc = (
            results[2 * b]["outT"].astype(np.float32)
            + results[2 * b + 1]["outT"].astype(np.float32)
        )
        # [P, NTO, T] -> [C, T] -> [T, C]
        out[b] = acc.transpose(1, 0, 2).reshape(C, T).T
    return out


_NC_CACHE = {}


def kernel(x, w_attn, b_attn, w_proj, b_proj):
    if "nc" not in _NC_CACHE:
        _NC_CACHE["nc"] = build_nc()
    nc = _NC_CACHE["nc"]
    in_maps = shard_inputs(x, w_attn, b_attn, w_proj, b_proj)
    res = run_bass_kernel_spmd(nc, in_maps, core_ids=list(range(8)))
    return unshard_output(res.results)


# revision 18
# speedup vs baseline: 1.1011x; 1.0321x over previous
"""Causal self-attention kernel for 8 Trainium2 NeuronCores.

Problem: B=4, T=2048, C=1024, H=16 heads (D=64).
Sharding: data-parallel over batch (4) x tensor-parallel over heads (2 groups
of 8 heads). Core c handles batch c//2, head-group c%2. Each core computes
qkv for its 8 heads, full causal attention on TxT scores, and its partial
projection output; the host sums the two head-group partials per batch.

v2 layout/schedule notes (per core):
  - i-chunk-outer pipeline: for each 512-token chunk ic: load x chunk,
    compute v tiles + q/k for ALL head pairs, then attention for all pairs
    at i-chunk ic (causal j <= i), then deferred normalization for ic and
    the projection for chunk ic-1 (one chunk behind, so the PE never waits
    on the normalization chain).
  - causal mask applied as a 0/1 bf16 multiply on the exp() output of the
    diagonal j-tile (cheap DVE op in 2x mode) instead of a -1e30 PSUM add.
  - softmax denominator l comes from a ones-column in v (PV matmul row 64);
    l rows are staged to SBUF, lane-spread with one DMA per chunk,
    inverted with reciprocal_approx_fast, cast to bf16, partition-broadcast
    with stride-0-source DMAs, and multiplied into yU in bf16.
  - projection bias+copy runs on the Scalar engine (per-partition bias),
    keeping DVE free; all steady-state DMAs are issued on the sync queue so
    the Scalar sequencer never stalls the exp stream.
  - all matmuls run in bf16 with fp32 PSUM accumulation; score pairs use
    tile_position row-packing (concurrent K=64 matmuls).
"""

import sys

if "/opt/trn_rl_repo" not in sys.path:
    sys.path.insert(0, "/opt/trn_rl_repo")

from contextlib import ExitStack

import ml_dtypes
import numpy as np

import concourse.bass as bass
import concourse.mybir as mybir
import concourse.tile as tile
from concourse.bass_utils import run_bass_kernel_spmd

BF16 = mybir.dt.bfloat16
F32 = mybir.dt.float32
NP_BF16 = ml_dtypes.bfloat16

P = 128
B, T, C = 4, 2048, 1024
H = 16
D = 64
HL = 8            # heads per core
NPAIR = HL // 2   # head pairs per core
NL = HL * D       # 512: local qkv width
CT = C // P       # 8 contraction tiles over C
DT = NL // P      # 4 contraction tiles over local head dims
NTO = C // P      # 8 output tiles for proj
TCH = T // 512    # 4 t-chunks
NJT = T // P      # 16 j tiles


def _split_excess_waits(nc, limit=1):
    """This walrus build supports a single sem-wait per instruction; move
    excess waits emitted by Tile onto preceding same-engine NoOps."""
    n = 0
    for bb in nc.main_func.blocks:
        out = []
        changed = False
        for inst in bb.instructions:
            si = inst.sync_info
            if si is not None and len(si.on_wait) > limit:
                waits = list(si.on_wait)
                excess, keep = waits[:-limit], waits[-limit:]
                for i in range(0, len(excess), limit):
                    out.append(
                        mybir.InstNoOp(
                            name=f"waitsplit_{n}",
                            ins=[],
                            outs=[],
                            engine=inst.engine,
                            sync_info=mybir.SyncInfo(
                                on_wait=excess[i : i + limit], on_update=[]
                            ),
                        )
                    )
                    n += 1
                si.on_wait = keep
                changed = True
            out.append(inst)
        if changed:
            bb.instructions = out
    return n


def build_nc(split_waits=True):
    nc = bass.Bass()
    AF = mybir.ActivationFunctionType

    xT = nc.dram_tensor("xT", [P, TCH, CT, 512], BF16, kind="ExternalInput")
    wq = nc.dram_tensor("wq", [P, CT, NL], BF16, kind="ExternalInput")
    wk = nc.dram_tensor("wk", [P, CT, NL], BF16, kind="ExternalInput")
    wv = nc.dram_tensor("wv", [P, CT, NL], BF16, kind="ExternalInput")
    wp = nc.dram_tensor("wp", [P, DT, C], BF16, kind="ExternalInput")
    bq = nc.dram_tensor("bq", [P, NPAIR], F32, kind="ExternalInput")
    bk = nc.dram_tensor("bk", [P, NPAIR], F32, kind="ExternalInput")
    bv = nc.dram_tensor("bv", [P, NL], F32, kind="ExternalInput")
    bp = nc.dram_tensor("bp", [P, NTO], F32, kind="ExternalInput")
    msk = nc.dram_tensor("msk", [P, P], BF16, kind="ExternalInput")
    outT = nc.dram_tensor("outT", [P, NTO, T], BF16, kind="ExternalOutput")

    with tile.TileContext(nc) as tc, ExitStack() as ctx:
        persist = ctx.enter_context(tc.tile_pool(name="persist", bufs=1))
        spsum = ctx.enter_context(tc.tile_pool(name="spsum", bufs=2, space="PSUM"))
        ypsum = ctx.enter_context(tc.tile_pool(name="ypsum", bufs=2, space="PSUM"))
        work = ctx.enter_context(tc.tile_pool(name="work", bufs=3))
        nwork = ctx.enter_context(tc.tile_pool(name="nwork", bufs=2))

        # ---- persistent SBUF tensors ----
        qT = persist.tile([P, NPAIR, T], BF16)   # [2x64d, pair, t]
        kT = persist.tile([P, NPAIR, T], BF16)
        vA = persist.tile([P, NJT, HL, D + 1], BF16)  # [j, jt, head, d|ones]
        yU = persist.tile([P, DT, T], BF16)  # y.T pair-packed; normalized in place
        lst = persist.tile([P, 2 * NPAIR, 512], F32)  # l staging (row 64 only)
        lrow = persist.tile([P, 512], F32)   # lane-spread l: row 8*ic+2*pr+h
        linv = persist.tile([P, 512], F32)
        linvb = persist.tile([P, 512], BF16)
        linv0 = persist.tile([1, 4 * 8, 512], BF16)  # 1/l rows gathered at p0
        onesb = persist.tile([1, D], BF16)   # lhsT for PE partition-broadcast
        xs = persist.tile([P, TCH, CT, 512], BF16)
        wqs = persist.tile([P, CT, NL], BF16)
        wks = persist.tile([P, CT, NL], BF16)
        wvs = persist.tile([P, CT, NL], BF16)
        wps = persist.tile([P, DT, C], BF16)
        bqs = persist.tile([P, NPAIR], F32)
        bks = persist.tile([P, NPAIR], F32)
        bvs = persist.tile([P, NL], F32)
        bps = persist.tile([P, NTO], F32)
        msks = persist.tile([P, 1, P], BF16)

        # ---- input DMAs: only what block 0 needs up front (x chunk 0 +
        # wv/wq/wk + biases); later chunks and wp are issued inside the
        # pipeline so they don't steal HBM bandwidth from the startup ----
        nc.scalar.dma_start(msks[:, 0, :], msk[:])
        nc.scalar.dma_start(wvs[:], wv[:])
        nc.scalar.dma_start(bvs[:], bv[:])
        nc.scalar.dma_start(bqs[:], bq[:])
        nc.scalar.dma_start(bks[:], bk[:])
        nc.scalar.dma_start(bps[:], bp[:])

        nc.sync.dma_start(xs[:, 0, 0:4, :], xT[:, 0, 0:4, :])
        nc.sync.dma_start(xs[:, 0, 4:8, :], xT[:, 0, 4:8, :])
        nc.sync.dma_start(wqs[:], wq[:])

        nc.gpsimd.dma_start(wks[:], wk[:])

        nc.vector.memset(vA[:, :, :, D : D + 1], 1.0)
        nc.vector.memset(onesb[:], 1.0)

        # warm the PE clock gate + preload the Exp table while DMAs run;
        # the warmup reads msks so it only waits on that tiny transfer
        wps_ = spsum.tile([P, 2, 512], F32, tag="s")
        nc.scalar.activation(lst[0:1, 0, 0:2], msks[0:1, 0, 0:2], AF.Exp,
                             scale=1.0)
        for _ in range(88):
            nc.tensor.matmul(wps_[:, 0, 0:P], lhsT=msks[:, 0, :],
                             rhs=msks[:, 0, :], start=True, stop=True)

        def xsl(ct, t0, n):  # slice of xs covering [t0, t0+n) at c-tile ct
            tc_i, o = divmod(t0, 512)
            return xs[:, tc_i, ct, o : o + n]

        def emit_v(tt):
            ps = spsum.tile([P, 2, 512], F32, tag="s")
            for ct in range(CT):
                nc.tensor.matmul(
                    ps[:, 0, :],
                    lhsT=xsl(ct, tt * P, P),
                    rhs=wvs[:, ct, :],
                    start=(ct == 0),
                    stop=(ct == CT - 1),
                )
            nc.vector.tensor_tensor(
                out=vA[:, tt, :, 0:D],
                in0=ps[:, 0, :].rearrange("p (h d) -> p h d", h=HL),
                in1=bvs.rearrange("p (h d) -> p h d", h=HL),
                op=mybir.AluOpType.add,
            )

        def emit_qk(nt, tc_i):
            ps = spsum.tile([P, 2, 512], F32, tag="s")
            t_sl = slice(tc_i * 512, (tc_i + 1) * 512)
            for ct in range(CT):
                nc.tensor.matmul(
                    ps[:, 0, :],
                    lhsT=wqs[:, ct, nt * P : (nt + 1) * P],
                    rhs=xs[:, tc_i, ct, :],
                    start=(ct == 0),
                    stop=(ct == CT - 1),
                )
            for ct in range(CT):
                nc.tensor.matmul(
                    ps[:, 1, :],
                    lhsT=wks[:, ct, nt * P : (nt + 1) * P],
                    rhs=xs[:, tc_i, ct, :],
                    start=(ct == 0),
                    stop=(ct == CT - 1),
                )
            nc.scalar.activation(
                qT[:, nt, t_sl], ps[:, 0, :], AF.Identity,
                bias=bqs[:, nt : nt + 1], scale=1.0,
            )
            nc.scalar.activation(
                kT[:, nt, t_sl], ps[:, 1, :], AF.Identity,
                bias=bks[:, nt : nt + 1], scale=1.0,
            )

        def emit_attention(pr, ic):
            hA, hB = 2 * pr, 2 * pr + 1
            njt = 4 * ic + 4  # causal: j tiles 0 .. 4*ic+3
            i0 = ic * 512
            yA = ypsum.tile([D + 1, 512], F32, tag="yA")
            yB = ypsum.tile([D + 1, 512], F32, tag="yB")
            sts = {}

            def emit_scores(jt):
                st = spsum.tile([P, 2, 512], F32, tag="s")
                sts[jt] = st
                ow = max(0, jt * P - i0)
                j_sl = slice(jt * P, (jt + 1) * P)
                i_sl = slice(i0 + ow, i0 + 512)
                nc.tensor.matmul(
                    st[:, 0, ow:512],
                    lhsT=kT[0:D, pr, j_sl],
                    rhs=qT[0:D, pr, i_sl],
                    start=True, stop=True,
                    tile_position=(0, 0),
                )
                nc.tensor.matmul(
                    st[:, 1, ow:512],
                    lhsT=kT[D:P, pr, j_sl],
                    rhs=qT[D:P, pr, i_sl],
                    start=True, stop=True,
                    tile_position=(64, 0),
                )

            emit_scores(0)
            if njt > 1:
                emit_scores(1)
            for jt in range(njt):
                st = sts.pop(jt)
                ow = max(0, jt * P - i0)
                pt = work.tile([P, 2, 512], BF16, tag="p")
                nc.scalar.activation(
                    pt[:, :, ow:512], st[:, :, ow:512], AF.Exp, scale=0.125
                )
                if jt >= 4 * ic:  # diagonal tile: zero above-diag via 0/1 mask
                    nc.vector.tensor_tensor(
                        out=pt[:, :, ow : ow + P],
                        in0=pt[:, :, ow : ow + P],
                        in1=msks[:].to_broadcast([P, 2, P]),
                        op=mybir.AluOpType.mult,
                    )
                if jt + 2 < njt:
                    emit_scores(jt + 2)
                nc.tensor.matmul(
                    yA[:, ow:512],
                    lhsT=vA[:, jt, hA, :],
                    rhs=pt[:, 0, ow:512],
                    start=(jt == 0),
                    stop=(jt == njt - 1),
                )
                nc.tensor.matmul(
                    yB[:, ow:512],
                    lhsT=vA[:, jt, hB, :],
                    rhs=pt[:, 1, ow:512],
                    start=(jt == 0),
                    stop=(jt == njt - 1),
                )
            # stage l rows (fp32, partition 64) and the unnormalized y (bf16)
            i_sl = slice(i0, i0 + 512)
            nc.vector.tensor_copy(lst[D : D + 1, 2 * pr, :], yA[D : D + 1, :])
            nc.vector.tensor_copy(lst[D : D + 1, 2 * pr + 1, :], yB[D : D + 1, :])
            nc.vector.tensor_copy(yU[0:D, pr, i_sl], yA[0:D, :])
            nc.vector.tensor_copy(yU[D:P, pr, i_sl], yB[0:D, :])

        def emit_spread(ic):
            # lane-spread the 8 staged l rows, invert, cast to bf16
            # (rows 32*ic.. to keep engine base partitions 32-aligned)
            r0 = 32 * ic
            nc.sync.dma_start(lrow[r0 : r0 + 8, :], lst[D : D + 1, :, :])
            nc.vector.reciprocal(linv[r0 : r0 + 8, :], lrow[r0 : r0 + 8, :])
            nc.vector.tensor_copy(linvb[r0 : r0 + 8, :], linv[r0 : r0 + 8, :])
            nc.sync.dma_start(
                linv0[0:1, 8 * ic : 8 * ic + 8, :], linvb[r0 : r0 + 8, :]
            )

        def emit_apply(ic):
            # PE (ones-column x row) partition-broadcasts 1/l into PSUM and
            # DVE multiplies y_u in place.
            i_sl = slice(ic * 512, (ic + 1) * 512)
            for pr in range(NPAIR):
                rA = 8 * ic + 2 * pr
                lbA = ypsum.tile([D, 512], F32, tag="yA")
                lbB = ypsum.tile([D, 512], F32, tag="yB")
                nc.tensor.matmul(
                    lbA[:], lhsT=onesb[:], rhs=linv0[0:1, rA, :],
                    start=True, stop=True,
                )
                nc.tensor.matmul(
                    lbB[:], lhsT=onesb[:], rhs=linv0[0:1, rA + 1, :],
                    start=True, stop=True,
                )
                nc.vector.tensor_tensor(
                    out=yU[0:D, pr, i_sl], in0=yU[0:D, pr, i_sl], in1=lbA[:],
                    op=mybir.AluOpType.mult,
                )
                nc.vector.tensor_tensor(
                    out=yU[D:P, pr, i_sl], in0=yU[D:P, pr, i_sl], in1=lbB[:],
                    op=mybir.AluOpType.mult,
                )

        def emit_proj(tc_i, nts=range(NTO), on_act=True):
            t_sl = slice(tc_i * 512, (tc_i + 1) * 512)
            for nt in nts:
                ps = spsum.tile([P, 2, 512], F32, tag="s")
                for dt in range(DT):
                    nc.tensor.matmul(
                        ps[:, 0, :],
                        lhsT=wps[:, dt, nt * P : (nt + 1) * P],
                        rhs=yU[:, dt, t_sl],
                        start=(dt == 0),
                        stop=(dt == DT - 1),
                    )
                ot = work.tile([P, 512], BF16, tag="out")
                if on_act:
                    nc.scalar.activation(
                        ot[:], ps[:, 0, :], AF.Identity,
                        bias=bps[:, nt : nt + 1], scale=1.0,
                    )
                else:
                    nc.vector.tensor_scalar(
                        out=ot[:], in0=ps[:, 0, :],
                        scalar1=bps[:, nt : nt + 1], scalar2=None,
                        op0=mybir.AluOpType.add,
                    )
                nc.sync.dma_start(outT[:, nt, t_sl], ot[:])

        # ---- main pipeline: chunk-outer with fine-grained interleave.
        # Block ic emits attention for chunk ic alternating with v/qk for
        # chunk ic+1 (fills PE during exp waits, keeps the clock gate warm);
        # normalization-apply and proj run one chunk behind. ----
        nc.sync.dma_start(xs[:, 1, 0:4, :], xT[:, 1, 0:4, :])
        nc.sync.dma_start(xs[:, 1, 4:8, :], xT[:, 1, 4:8, :])
        for tt in range(4):
            emit_v(tt)
        for pr in range(NPAIR):
            emit_qk(pr, 0)
        for ic in range(TCH):
            nxt = ic + 1
            if ic == 0:  # prefetch chunk 2 + proj weights during block 0
                nc.sync.dma_start(xs[:, 2, 0:4, :], xT[:, 2, 0:4, :])
                nc.sync.dma_start(xs[:, 2, 4:8, :], xT[:, 2, 4:8, :])
                nc.gpsimd.dma_start(wps[:], wp[:])
            elif ic == 1:  # prefetch chunk 3 during block 1
                nc.gpsimd.dma_start(xs[:, 3, 0:4, :], xT[:, 3, 0:4, :])
                nc.gpsimd.dma_start(xs[:, 3, 4:8, :], xT[:, 3, 4:8, :])
            for pr in range(NPAIR):
                emit_attention(pr, ic)
                if nxt < TCH:
                    emit_v(4 * nxt + pr)
                    emit_qk(pr, nxt)
            if ic > 0:
                emit_apply(ic - 1)
                emit_proj(ic - 1)
            emit_spread(ic)
        emit_apply(TCH - 1)
        emit_proj(TCH - 1)

    if split_waits:
        _split_excess_waits(nc, 1)
    return nc


def shard_inputs(x, w_attn, b_attn, w_proj, b_proj):
    """Build the 8 per-core input dicts (core = 2*batch + head_group)."""
    x = np.asarray(x, dtype=np.float32)
    w_attn = np.asarray(w_attn, dtype=np.float32)
    b_attn = np.asarray(b_attn, dtype=np.float32)
    w_proj = np.asarray(w_proj, dtype=np.float32)
    b_proj = np.asarray(b_proj, dtype=np.float32)

    # 0/1 multiplicative causal mask for a diagonal 128x128 block of
    # S.T ([j, i]): 1 where j <= i, 0 above the diagonal.
    pp = np.arange(P)
    msk = np.where(pp[:, None] <= pp[None, :], 1.0, 0.0).astype(NP_BF16)

    def wtile(w2d, ncols):  # [C_rows, ncols] -> [P, rows//P, ncols] bf16
        r = w2d.shape[0]
        return np.ascontiguousarray(
            w2d.reshape(r // P, P, ncols).transpose(1, 0, 2)
        ).astype(NP_BF16)

    in_maps = []
    for core in range(8):
        b, hg = divmod(core, 2)
        q0 = hg * NL
        xt = np.ascontiguousarray(x[b].T)  # [C, T]
        m = {
            "xT": np.ascontiguousarray(
                xt.reshape(CT, P, TCH, 512).transpose(1, 2, 0, 3)
            ).astype(NP_BF16),
            "wq": wtile(w_attn[:, q0 : q0 + NL], NL),
            "wk": wtile(w_attn[:, C + q0 : C + q0 + NL], NL),
            "wv": wtile(w_attn[:, 2 * C + q0 : 2 * C + q0 + NL], NL),
            "wp": wtile(w_proj[q0 : q0 + NL, :], C),
            "bq": np.ascontiguousarray(
                b_attn[q0 : q0 + NL].reshape(NPAIR, P).T
            ).astype(np.float32),
            "bk": np.ascontiguousarray(
                b_attn[C + q0 : C + q0 + NL].reshape(NPAIR, P).T
            ).astype(np.float32),
            "bv": np.broadcast_to(
                b_attn[2 * C + q0 : 2 * C + q0 + NL], (P, NL)
            ).astype(np.float32),
            "bp": (
                np.ascontiguousarray(b_proj.reshape(NTO, P).T).astype(np.float32)
                if hg == 0
                else np.zeros((P, NTO), np.float32)
            ),
            "msk": msk,
        }
        in_maps.append(m)
    return in_maps


def unshard_output(results):
    """Combine 8 per-core outT [P, NTO, T] partials into [B, T, C] fp32."""
    out = np.empty((B, T, C), dtype=np.float32)
    for b in range(B):
        acc = (
            results[2 * b]["outT"].astype(np.float32)
            + results[2 * b + 1]["outT"].astype(np.float32)
        )
        # [P, NTO, T] -> [C, T] -> [T, C]
        out[b] = acc.transpose(1, 0, 2).reshape(C, T).T
    return out


_NC_CACHE = {}


def kernel(x, w_attn, b_attn, w_proj, b_proj):
    if "nc" not in _NC_CACHE:
        _NC_CACHE["nc"] = build_nc()
    nc = _NC_CACHE["nc"]
    in_maps = shard_inputs(x, w_attn, b_attn, w_proj, b_proj)
    res = run_bass_kernel_spmd(nc, in_maps, core_ids=list(range(8)))
    return unshard_output(res.results)
